# revision 4
# baseline (speedup 1.0000x reference)
"""Trainium2 Bass kernel for nn_BinarizedVGG19_13924283974418.

The reference network is a binarized VGG19 forward pass where every layer
computes  relu(conv(ste_sign(x), ste_sign(w)) + b)  with ste_sign(x>=0)=+1.
Because every layer input after layer 0 is a ReLU output (>= 0 everywhere),
ste_sign of it is identically +1: layers 1..14 only feed +1s forward, and the
whole network output equals

    maxpool2x2( relu( b15 + T ) )            broadcast over the batch,

where T[h,w,co] on the final 14x14 grid sums S[ky,kx,co] over the in-bounds
conv taps at (h,w), and S[ky,kx,co] = sum_cin sign(w15[ky,kx,cin,co]).
This identity holds for every input and is bitwise-exact in f32 (all conv
intermediates are small integers).

Sharding: pure output-channel data parallelism — each of the 8 cores reduces
its own 64-channel slice of w15 (1.18 MB of the 9.4 MB weight read per core),
computes its [8,7,7,64] output shard on-device, and the host concatenates the
shards along the channel axis.  No collectives.

Per-core device program:
  1. DMA the [512,3,3,64] weight slice (host pre-transposed so each SBUF
     partition reads contiguous runs) into SBUF as 4 cin-chunks of 128.
  2. ge = (w >= 0) as bf16 {0,1}  (DVE/GPSIMD tensor_scalar is_ge — exact
     ste_sign semantics incl. sign(0)=+1).
  3. Per conv tap t: count[t,co] = ones[128]^T @ ge-chunk, accumulated over
     the 4 chunks in PSUM  ->  S' = 2*count - 512 per tap (folded below).
  4. One tiny matmul with a constant [10,9] matrix turns the 9 tap counts
     (+ a constant row) into the 9 boundary-region conv values
     T = 2*count_sum - 512*ntaps  (exact integers).
  5. y = max(T + b, 0) (single f32 rounding, matches the reference exactly),
     then the 2x2 maxpool collapses to region-wise maxes (row classes x col
     classes), broadcast onto the 7x7 output map.
  6. PE-transpose [64,49] -> [49,64] (pure data movement, exact) and DMA the
     8 identical batch images out.
"""

import numpy as np

import concourse.bass as bass
import concourse.tile as tile
from concourse import bacc, mybir
from concourse.bass_utils import run_bass_kernel_spmd

N_CORES = 8
CIN = 512
CO = 64  # output-channel slice per core (512 / 8)
NCHUNK = CIN // 128

# conv tap ky (or kx) is in-bounds for row (col) class r: 0=first, 1=interior,
# 2=last of the 14x14 grid
_VALID = {0: (1, 2), 1: (0, 1, 2), 2: (0, 1)}


def _m9_ext() -> np.ndarray:
    """[10,9]: maps (9 tap counts + const 1) -> T = 2*count_sum - 512*ntaps
    for the 9 (row class, col class) regions; column index rc = r*3 + c."""
    m = np.zeros((10, 9), np.float32)
    for r in range(3):
        for c in range(3):
            rc = r * 3 + c
            n = 0
            for ky in _VALID[r]:
                for kx in _VALID[c]:
                    m[ky * 3 + kx, rc] = 2.0
                    n += 1
            m[9, rc] = -512.0 * n
    return m


def _build_nc():
    nc = bacc.Bacc("TRN2", target_bir_lowering=False, debug=False)
    w_dram = nc.declare_dram_parameter("w", [CIN, 3, 3, CO], mybir.dt.float32, isOutput=False)
    b_dram = nc.declare_dram_parameter("b", [CO, 1], mybir.dt.float32, isOutput=False)
    m9_dram = nc.declare_dram_parameter("m9", [10, 9], mybir.dt.float32, isOutput=False)
    eye_dram = nc.declare_dram_parameter("eye", [CO, CO], mybir.dt.float32, isOutput=False)
    ones_row_dram = nc.declare_dram_parameter("onesrow", [1, CO], mybir.dt.float32, isOutput=False)
    out_dram = nc.declare_dram_parameter("out", [8, 7, 7, CO], mybir.dt.float32, isOutput=True)

    f32 = mybir.dt.float32
    bf16 = mybir.dt.bfloat16
    GE = mybir.AluOpType.is_ge
    ADD = mybir.AluOpType.add
    MAX = mybir.AluOpType.max

    with tile.TileContext(nc) as tc:
        with (
            tc.tile_pool(name="sbuf", bufs=1) as pool,
            tc.tile_pool(name="psum", bufs=1, space=bass.MemorySpace.PSUM) as psum,
        ):
            engs = [nc.vector, nc.gpsimd]

            # 1+2: weight load + binarize, pipelined per (cin-chunk, ky)
            w_sb = pool.tile([128, NCHUNK, 3, 3, CO], f32)
            ge_sb = pool.tile([128, NCHUNK, 3, 3, CO], bf16)
            for c in range(NCHUNK):
                for ky in range(3):
                    nc.sync.dma_start(w_sb[:, c, ky], w_dram[c * 128:(c + 1) * 128, ky])
                    engs[(c * 3 + ky) % 2].tensor_scalar(
                        out=ge_sb[:, c, ky], in0=w_sb[:, c, ky],
                        scalar1=0.0, scalar2=None, op0=GE)

            ones_sb = pool.tile([128, 1], bf16)
            nc.vector.memset(ones_sb[:], 1.0)
            eye_sb = pool.tile([CO, CO], f32)
            nc.sync.dma_start(eye_sb[:], eye_dram[:])

            # 3: per-tap cross-partition counts, accumulated over cin chunks
            # (PE matmul outputs must start at partition 0, so produce
            # [co, tap] and PE-transpose — exact data movement — to [tap, co])
            sA_psum = psum.tile([CO, 9], f32)
            for t in range(9):
                ky, kx = divmod(t, 3)
                for c in range(NCHUNK):
                    nc.tensor.matmul(
                        sA_psum[:, t:t + 1], ge_sb[:, c, ky, kx, :], ones_sb[:, :],
                        start=(c == 0), stop=(c == NCHUNK - 1))
            sA_sb = pool.tile([CO, 9], f32)
            nc.vector.tensor_copy(sA_sb[:], sA_psum[:])
            sT_psum = psum.tile([9, CO], f32)
            nc.tensor.transpose(sT_psum[:], sA_sb[:], eye_sb[:])

            s_sb = pool.tile([10, CO], f32)
            nc.vector.tensor_copy(s_sb[0:9, :], sT_psum[:, :])
            nc.sync.dma_start(s_sb[9:10, :], ones_row_dram[:])

            # 4: region conv values T (exact small-integer matmul)
            m9_sb = pool.tile([10, 9], f32)
            nc.sync.dma_start(m9_sb[:], m9_dram[:])
            t9_psum = psum.tile([CO, 3, 3], f32)
            nc.tensor.matmul(t9_psum[:], s_sb[:, :], m9_sb[:, :], start=True, stop=True)

            # 5: y = max(T + b, 0); then maxpool region algebra
            b_sb = pool.tile([CO, 1], f32)
            nc.sync.dma_start(b_sb[:], b_dram[:])
            y3 = pool.tile([CO, 3, 3], f32)
            nc.vector.tensor_scalar(
                out=y3[:], in0=t9_psum[:], scalar1=b_sb[:, :], scalar2=0.0,
                op0=ADD, op1=MAX)

            u3 = pool.tile([CO, 3, 3], f32)  # pooled-row classes
            v3 = pool.tile([CO, 3, 3], f32)  # pooled-row x pooled-col classes
            nc.vector.tensor_max(u3[:, 0], y3[:, 0], y3[:, 1])
            nc.vector.tensor_copy(u3[:, 1], y3[:, 1])
            nc.vector.tensor_max(u3[:, 2], y3[:, 1], y3[:, 2])
            nc.vector.tensor_max(v3[:, :, 0], u3[:, :, 0], u3[:, :, 1])
            nc.vector.tensor_copy(v3[:, :, 1], u3[:, :, 1])
            nc.vector.tensor_max(v3[:, :, 2], u3[:, :, 1], u3[:, :, 2])

            # broadcast the 9 region values onto the 7x7 map
            out49 = pool.tile([CO, 7, 7], f32)
            nc.vector.memset(out49[:], 0.0)
            R = ((0, 1), (1, 6), (6, 7))
            for a in range(3):
                for b in range(3):
                    reg = out49[:, R[a][0]:R[a][1], R[b][0]:R[b][1]]
                    engs[(a * 3 + b) % 2].tensor_scalar(
                        out=reg, in0=reg, scalar1=v3[:, a, b:b + 1],
                        scalar2=None, op0=ADD)

            # 6: transpose to [pixel, channel] and write the 8 batch images
            outT_psum = psum.tile([49, CO], f32)
            nc.tensor.transpose(outT_psum[:], out49[:], eye_sb[:])
            out_sb = pool.tile([49, CO], f32)
            nc.vector.tensor_copy(out_sb[:], outT_psum[:])
            for img in range(8):
                nc.sync.dma_start(out_dram[img].rearrange("h w c -> (h w) c"), out_sb[:])

    nc.compile()
    return nc


_CACHE = {}


def _get_nc():
    if "nc" not in _CACHE:
        _CACHE["nc"] = _build_nc()
    return _CACHE["nc"]


def _in_maps(ws, bs):
    w15 = np.asarray(ws[15], dtype=np.float32)  # [3,3,512,512]
    b15 = np.asarray(bs[15], dtype=np.float32)  # [512]
    m9 = _m9_ext()
    eye = np.eye(CO, dtype=np.float32)
    maps = []
    for k in range(N_CORES):
        sl = slice(k * CO, (k + 1) * CO)
        wk = np.ascontiguousarray(np.transpose(w15[:, :, :, sl], (2, 0, 1, 3)))
        bk = np.ascontiguousarray(b15[sl].reshape(CO, 1))
        maps.append({"w": wk, "b": bk, "m9": m9, "eye": eye,
                     "onesrow": np.ones((1, CO), np.float32)})
    return maps


LAST_RESULT = None


def kernel(inputs=None, ws=None, bs=None, _trace=False):
    global LAST_RESULT
    nc = _get_nc()
    res = run_bass_kernel_spmd(nc, _in_maps(ws, bs), list(range(N_CORES)), trace=_trace)
    LAST_RESULT = res
    return np.concatenate([res.results[k]["out"] for k in range(N_CORES)], axis=-1)


# revision 7
# speedup vs baseline: 1.8717x; 1.8717x over previous
"""Trainium2 Bass kernel for nn_BinarizedVGG19_13924283974418.

The reference network is a binarized VGG19 forward pass where every layer
computes  relu(conv(ste_sign(x), ste_sign(w)) + b)  with ste_sign(x>=0)=+1.
Because every layer input after layer 0 is a ReLU output (>= 0 everywhere),
ste_sign of it is identically +1: layers 1..14 only feed +1s forward, and the
whole network output equals

    maxpool2x2( relu( b15 + T ) )            broadcast over the batch,

where T[h,w,co] on the final 14x14 grid sums S[ky,kx,co] over the in-bounds
conv taps at (h,w), and S[ky,kx,co] = sum_cin sign(w15[ky,kx,cin,co]).
This identity holds for every input and is bitwise-exact in f32 (all conv
intermediates are small integers, and max/relu/+bias commute monotonically).

Sharding: pure output-channel data parallelism — each of the 8 cores reduces
its own 64-channel slice of w15 (1.18 MB of the 9.4 MB weight read per core),
computes its [8,7,7,64] output shard on-device, and the host concatenates the
shards along the channel axis.  No collectives.

Per-core device program:
  1. Four [128, 2304] weight DMAs (one per cin-chunk, 9 KB contiguous per
     partition), issued from four different engines so descriptor generation
     runs in parallel.
  2. ge = (w >= 0) as bf16 {0,1} on VectorE only (exact ste_sign semantics
     incl. sign(0)=+1; GPSIMD is avoided — it is ~12x slower on this op and
     its shared SBUF port stalls the DVE).
  3. 37 accumulating matmuls produce, directly in one [64, 9] PSUM tile,
     T[co, region] = 2*count_sum - 512*ntaps for the 9 boundary regions:
     per (chunk, tap), lhsT = ge-slice [128cin, 64co], rhs = that tap's
     inclusion row of m9 (x2.0) replicated across cin; one extra matmul with
     an all-ones lhsT adds the exact -512*ntaps constant.  All products and
     partial sums are small integers => exact in bf16 x f32-PSUM.
  4. The 2x2 maxpool collapses to region maxes (row classes x col classes):
     6 small DVE ops on the integer T values.
  5. PE-transpose [64,9] -> [9,64] (pure data movement), broadcast onto the
     49-pixel output map with a {0,1} matmul (exact: one term per pixel),
     then y = max(T + b, 0) — the single f32 rounding matches the reference.
  6. One DVE broadcast-copy replicates the [49, 64] map for the 8 identical
     batch images; a single DMA writes the [8,7,7,64] shard.
"""

import numpy as np

import concourse.bass as bass
import concourse.tile as tile
from concourse import bacc, mybir
from concourse.bass_utils import run_bass_kernel_spmd

N_CORES = 8
CIN = 512
CO = 64  # output-channel slice per core (512 / 8)
NCHUNK = CIN // 128

# conv tap ky (or kx) is in-bounds for row (col) class r: 0=first, 1=interior,
# 2=last of the 14x14 grid
_VALID = {0: (1, 2), 1: (0, 1, 2), 2: (0, 1)}


def _m9_ext() -> np.ndarray:
    """[10,9] f32: rows 0..8 map tap counts -> 2*count_sum per region
    (rc = r*3 + c); row 9 is the per-cin share of -512*ntaps (i.e. -4*ntaps,
    summed over 128 cin partitions by an all-ones matmul)."""
    m = np.zeros((10, 9), np.float32)
    for r in range(3):
        for c in range(3):
            rc = r * 3 + c
            n = 0
            for ky in _VALID[r]:
                for kx in _VALID[c]:
                    m[ky * 3 + kx, rc] = 2.0
                    n += 1
            m[9, rc] = -4.0 * n
    return m


def _b49() -> np.ndarray:
    """[9,49] f32 {0,1}: region (a*3+b) -> pixels of the 7x7 pooled map."""
    b = np.zeros((9, 49), np.float32)
    rcls = [0] + [1] * 5 + [2]
    for i in range(7):
        for j in range(7):
            b[rcls[i] * 3 + rcls[j], i * 7 + j] = 1.0
    return b


def _build_nc():
    nc = bacc.Bacc("TRN2", target_bir_lowering=False, debug=False)
    f32 = mybir.dt.float32
    bf16 = mybir.dt.bfloat16
    GE = mybir.AluOpType.is_ge
    ADD = mybir.AluOpType.add
    MAX = mybir.AluOpType.max

    w_dram = nc.declare_dram_parameter("w", [CIN, 3, 3, CO], f32, isOutput=False)
    m9_dram = nc.declare_dram_parameter("m9rep", [128, 10, 9], bf16, isOutput=False)
    eye_dram = nc.declare_dram_parameter("eye", [CO, CO], f32, isOutput=False)
    b49_dram = nc.declare_dram_parameter("b49", [9, 49], f32, isOutput=False)
    btile_dram = nc.declare_dram_parameter("btile", [49, CO], f32, isOutput=False)
    out_dram = nc.declare_dram_parameter("out", [8, 7, 7, CO], f32, isOutput=True)

    with tile.TileContext(nc) as tc:
        with (
            tc.tile_pool(name="sbuf", bufs=1) as pool,
            tc.tile_pool(name="psum", bufs=1, space=bass.MemorySpace.PSUM) as psum,
        ):
            # 1: weight load, one DMA per cin-chunk on four engines in parallel
            w_sb = pool.tile([128, NCHUNK, 3, 3, CO], f32)
            dma_engs = [nc.sync, nc.scalar, nc.gpsimd, nc.sync]
            for c in range(NCHUNK):
                dma_engs[c].dma_start(
                    w_sb[:, c], w_dram[c * 128:(c + 1) * 128])

            # prefetch constants (after the big DMAs in issue order)
            m9_sb = pool.tile([128, 10, 9], bf16)
            nc.scalar.dma_start(m9_sb[:], m9_dram[:])
            eye_sb = pool.tile([CO, CO], f32)
            nc.gpsimd.dma_start(eye_sb[:], eye_dram[:])
            b49_sb = pool.tile([9, 49], f32)
            nc.scalar.dma_start(b49_sb[:], b49_dram[:])
            btile_sb = pool.tile([49, CO], f32)
            nc.gpsimd.dma_start(btile_sb[:], btile_dram[:])
            ones_sb = pool.tile([128, CO], bf16)
            nc.vector.memset(ones_sb[:], 1.0)

            # 2: binarize (VectorE only)
            ge_sb = pool.tile([128, NCHUNK, 3, 3, CO], bf16)
            for c in range(NCHUNK):
                nc.vector.tensor_scalar(
                    out=ge_sb[:, c], in0=w_sb[:, c],
                    scalar1=0.0, scalar2=None, op0=GE)

            # 3: 37 accumulating matmuls -> T[co, region] (exact integers)
            t9_psum = psum.tile([CO, 3, 3], f32)
            first = True
            for c in range(NCHUNK):
                for t in range(9):
                    ky, kx = divmod(t, 3)
                    nc.tensor.matmul(
                        t9_psum[:], ge_sb[:, c, ky, kx, :], m9_sb[:, t, :],
                        start=first, stop=False)
                    first = False
            nc.tensor.matmul(
                t9_psum[:], ones_sb[:], m9_sb[:, 9, :], start=False, stop=True)

            # 4: maxpool region algebra on integer T
            t9_sb = pool.tile([CO, 3, 3], f32)
            nc.vector.tensor_copy(t9_sb[:], t9_psum[:])
            u3 = pool.tile([CO, 3, 3], f32)  # pooled-row classes
            v3 = pool.tile([CO, 3, 3], f32)  # pooled-row x pooled-col classes
            nc.vector.tensor_max(u3[:, 0], t9_sb[:, 0], t9_sb[:, 1])
            nc.vector.tensor_copy(u3[:, 1], t9_sb[:, 1])
            nc.vector.tensor_max(u3[:, 2], t9_sb[:, 1], t9_sb[:, 2])
            nc.vector.tensor_max(v3[:, :, 0], u3[:, :, 0], u3[:, :, 1])
            nc.vector.tensor_copy(v3[:, :, 1], u3[:, :, 1])
            nc.vector.tensor_max(v3[:, :, 2], u3[:, :, 1], u3[:, :, 2])

            # 5: transpose regions to partitions, broadcast to 7x7, bias+relu
            vT_psum = psum.tile([9, CO], f32)
            nc.tensor.transpose(vT_psum[:], v3[:], eye_sb[:])
            vT_sb = pool.tile([9, CO], f32)
            nc.vector.tensor_copy(vT_sb[:], vT_psum[:])
            out49_psum = psum.tile([49, CO], f32)
            nc.tensor.matmul(out49_psum[:], b49_sb[:], vT_sb[:], start=True, stop=True)
            y49_sb = pool.tile([49, CO], f32)
            nc.vector.tensor_tensor(out=y49_sb[:], in0=out49_psum[:], in1=btile_sb[:], op=ADD)
            nc.vector.tensor_scalar(out=y49_sb[:], in0=y49_sb[:], scalar1=0.0, scalar2=None, op0=MAX)

            # 6: replicate for the 8 identical batch images, single DMA out
            out8_sb = pool.tile([49, 8, CO], f32)
            nc.vector.tensor_copy(out8_sb[:], y49_sb[:, None, :].broadcast_to([49, 8, CO]))
            nc.sync.dma_start(out_dram.rearrange("b h w c -> (h w) b c"), out8_sb[:])

    nc.compile()
    return nc


_CACHE = {}


def _get_nc():
    if "nc" not in _CACHE:
        _CACHE["nc"] = _build_nc()
    return _CACHE["nc"]


def _in_maps(ws, bs):
    w15 = np.asarray(ws[15], dtype=np.float32)  # [3,3,512,512]
    b15 = np.asarray(bs[15], dtype=np.float32)  # [512]
    bf16_np = mybir.dt.np(mybir.dt.bfloat16)
    m9rep = np.ascontiguousarray(
        np.broadcast_to(_m9_ext()[None], (128, 10, 9))).astype(bf16_np)
    eye = np.eye(CO, dtype=np.float32)
    b49 = _b49()
    maps = []
    for k in range(N_CORES):
        sl = slice(k * CO, (k + 1) * CO)
        wk = np.ascontiguousarray(np.transpose(w15[:, :, :, sl], (2, 0, 1, 3)))
        btile = np.ascontiguousarray(np.broadcast_to(b15[sl][None], (49, CO)))
        maps.append({"w": wk, "m9rep": m9rep, "eye": eye, "b49": b49,
                     "btile": btile})
    return maps


LAST_RESULT = None


def kernel(inputs=None, ws=None, bs=None, _trace=False):
    global LAST_RESULT
    nc = _get_nc()
    res = run_bass_kernel_spmd(nc, _in_maps(ws, bs), list(range(N_CORES)), trace=_trace)
    LAST_RESULT = res
    return np.concatenate([res.results[k]["out"] for k in range(N_CORES)], axis=-1)
